# revision 6
# baseline (speedup 1.0000x reference)
"""Correlation-volume kernel for Trainium2 (8 NeuronCores, data-parallel over B).

corr[b, d, h, w] = sum_c L[b,h,w,c] * R[b,h,w-d,c], 0 <= d < 48, zero-padded w-d < 0.

Device strategy (per core = one batch):
  For each h row: transpose L/R rows into [C, W] layout on the PE (fp32r
  transpose-mode), then compute banded Gram tiles
      G^T[u, w] = sum_c R^T[c, u] * L^T[c, w]   (u-chunks of 128/128/64,
  moving = full 320-wide L^T row so fp32r streams at 1 cyc/row), copy the
  valid band window of each PSUM tile to SBUF and DMA rectangular blocks to
  DRAM. The host extracts the 48 diagonals (corr[d,h,w] = G^T[w-d, w]) while
  unsharding — that transpose is host-side glue, free for the device.
"""

import os
import sys

import numpy as np

for _p in (
    "/root/.axon_site",
    "/root/.axon_site/_ro/trn_rl_repo",
    "/root/.axon_site/_ro/pypackages",
    "/opt/trn_rl_repo",
    "/opt/pypackages",
):
    if os.path.isdir(_p) and _p not in sys.path:
        sys.path.append(_p)

import concourse.bacc as bacc
import concourse.mybir as mybir
import concourse.tile as tile
from concourse.bass_utils import run_bass_kernel_spmd

B, H, W, C, D = 8, 160, 320, 128, 48
NH = 10  # h rows per DMA batch
F32 = mybir.dt.float32
F32R = mybir.dt.float32r

# u-chunks: (u0, M) with output window w in [u0, u0 + M + D - 2] clipped to W
WTILES = [(0, 128), (128, 128), (256, 64)]
BLOCKS = [  # (name, u0, M, window_width)
    ("OutA", 0, 128, 175),
    ("OutB", 128, 128, 175),
    ("OutC", 256, 64, 64),
]

_cache: dict = {}


def _build(h_run: int = H):
    nc = bacc.Bacc("TRN2", target_bir_lowering=False, debug=False, num_devices=B)
    L = nc.dram_tensor("L", [H, W, C], F32, kind="ExternalInput").ap()
    R = nc.dram_tensor("R", [H, W, C], F32, kind="ExternalInput").ap()
    IDENT = nc.dram_tensor("IDENT", [128, 128], F32, kind="ExternalInput").ap()
    outs = {
        name: nc.dram_tensor(name, [H, M, Wn], F32, kind="ExternalOutput").ap()
        for name, _, M, Wn in BLOCKS
    }

    with tile.TileContext(nc) as tc:
        with (
            tc.tile_pool(name="const", bufs=1) as cpool,
            tc.tile_pool(name="loads", bufs=2) as lpool,
            tc.tile_pool(name="trans", bufs=3) as tpool,
            tc.tile_pool(name="outbuf", bufs=2) as opool,
            tc.tile_pool(name="pst", bufs=4, space="PSUM") as pst_pool,
            tc.tile_pool(name="psg", bufs=3, space="PSUM") as psg_pool,
        ):
            ident = cpool.tile([128, 128], F32R)
            nc.sync.dma_start(out=ident[:], in_=IDENT[:].bitcast(F32R))

            for hb in range(0, h_run, NH):
                nat = {}
                for ti, (w0, tw) in enumerate(WTILES):
                    for tname, src in (("L", L), ("R", R)):
                        t = lpool.tile([tw, NH, C], F32R, tag=f"nat{tname}{ti}")
                        nc.sync.dma_start(
                            out=t[:],
                            in_=src[hb : hb + NH, w0 : w0 + tw, :]
                            .rearrange("h w c -> w h c")
                            .bitcast(F32R),
                        )
                        nat[(tname, ti)] = t

                outt = {
                    name: opool.tile([M, NH, Wn], F32, tag=name, name=f"t_{name}")
                    for name, _, M, Wn in BLOCKS
                }

                for hl in range(NH):
                    trs = {}
                    for tname in ("L", "R"):
                        tt = tpool.tile([C, W], F32R, tag=f"T{tname}")
                        for ti, (w0, tw) in enumerate(WTILES):
                            ps = pst_pool.tile([C, 128], F32, tag="pst")
                            nc.tensor.transpose(
                                out=ps[:, :tw].bitcast(F32R),
                                in_=nat[(tname, ti)][:tw, hl, :],
                                identity=ident[:tw, :tw],
                            )
                            nc.vector.tensor_copy(
                                out=tt[:, w0 : w0 + tw], in_=ps[:, :tw]
                            )
                        trs[tname] = tt

                    for name, u0, M, Wn in BLOCKS:
                        pg = psg_pool.tile([128, W], F32, tag="psg")
                        nc.tensor.matmul(
                            out=pg[:M, :],
                            lhsT=trs["R"][:, u0 : u0 + M],
                            rhs=trs["L"][:],
                            start=True,
                            stop=True,
                        )
                        nc.vector.tensor_copy(
                            out=outt[name][:, hl, :], in_=pg[:M, u0 : u0 + Wn]
                        )

                for name, _, M, Wn in BLOCKS:
                    nc.sync.dma_start(
                        out=outs[name][hb : hb + NH].rearrange("h u j -> u h j"),
                        in_=outt[name][:],
                    )

    nc.compile()
    return nc


def _get_nc(h_run: int = H):
    if h_run not in _cache:
        _cache[h_run] = _build(h_run)
    return _cache[h_run]


def _reconstruct(results) -> np.ndarray:
    """Assemble [B, D, H, W] from the per-core band blocks."""
    XA = np.stack([r["OutA"] for r in results]).reshape(B, H, 128 * 175)
    XB = np.stack([r["OutB"] for r in results]).reshape(B, H, 128 * 175)
    XC = np.stack([r["OutC"] for r in results]).reshape(B, H, 64 * 64)
    out = np.zeros((B, D, H, W), np.float32)
    idx = np.arange(128) * 176
    for d in range(D):
        out[:, d, :, d : d + 128] = XA[:, :, idx + d]
        out[:, d, :, 128 + d : 256 + d] = XB[:, :, idx + d]
        nc_ = 64 - d
        out[:, d, :, 256 + d :] = XC[:, :, np.arange(nc_) * 65 + d]
    return out


def _run(L_full, R_full, h_run: int = H, trace: bool = False):
    L_full = np.ascontiguousarray(np.asarray(L_full), dtype=np.float32)
    R_full = np.ascontiguousarray(np.asarray(R_full), dtype=np.float32)
    assert L_full.shape == (B, H, W, C), L_full.shape
    nc = _get_nc(h_run)
    eye = np.eye(128, dtype=np.float32)
    in_maps = [{"L": L_full[b], "R": R_full[b], "IDENT": eye} for b in range(B)]
    res = run_bass_kernel_spmd(
        nc, in_maps, list(range(B)), trace=trace, trace_cores=[0] if trace else None
    )
    return _reconstruct(res.results), res


def kernel(L_corr, R_corr):
    out, _ = _run(L_corr, R_corr)
    return out


# revision 7
# speedup vs baseline: 1.4493x; 1.4493x over previous
"""Correlation-volume kernel for Trainium2 (8 NeuronCores, data-parallel over B).

corr[b, d, h, w] = sum_c L[b,h,w,c] * R[b,h,w-d,c], 0 <= d < 48, zero-padded w-d < 0.

Device strategy (per core = one batch):
  For each h row: transpose L/R rows into [C, W] layout on the PE (fp32r
  transpose-mode), then compute banded Gram tiles
      G^T[u, w] = sum_c R^T[c, u] * L^T[c, w]   (u-chunks of 128/128/64,
  moving = full 320-wide L^T row so fp32r streams at 1 cyc/row), copy the
  valid band window of each PSUM tile to SBUF and DMA rectangular blocks to
  DRAM. The host extracts the 48 diagonals (corr[d,h,w] = G^T[w-d, w]) while
  unsharding — that transpose is host-side glue, free for the device.
"""

import os
import sys

import numpy as np

for _p in (
    "/root/.axon_site",
    "/root/.axon_site/_ro/trn_rl_repo",
    "/root/.axon_site/_ro/pypackages",
    "/opt/trn_rl_repo",
    "/opt/pypackages",
):
    if os.path.isdir(_p) and _p not in sys.path:
        sys.path.append(_p)

import concourse.bacc as bacc
import concourse.mybir as mybir
import concourse.tile as tile
from concourse.bass_utils import run_bass_kernel_spmd

B, H, W, C, D = 8, 160, 320, 128, 48
NH = 10  # h rows per DMA batch
F32 = mybir.dt.float32
F16 = mybir.dt.float16

# u-chunks: (u0, M) with output window w in [u0, u0 + M + D - 2] clipped to W
WTILES = [(0, 128), (128, 128), (256, 64)]
BLOCKS = [  # (name, u0, M, window_width)
    ("OutA", 0, 128, 175),
    ("OutB", 128, 128, 175),
    ("OutC", 256, 64, 64),
]

_cache: dict = {}


def _build(h_run: int = H):
    nc = bacc.Bacc("TRN2", target_bir_lowering=False, debug=False, num_devices=B)
    L = nc.dram_tensor("L", [H, W, C], F32, kind="ExternalInput").ap()
    R = nc.dram_tensor("R", [H, W, C], F32, kind="ExternalInput").ap()
    IDENT = nc.dram_tensor("IDENT", [128, 128], F16, kind="ExternalInput").ap()
    outs = {
        name: nc.dram_tensor(name, [H, M, Wn], F32, kind="ExternalOutput").ap()
        for name, _, M, Wn in BLOCKS
    }

    with tile.TileContext(nc) as tc:
        with (
            tc.tile_pool(name="const", bufs=1) as cpool,
            tc.tile_pool(name="loads", bufs=2) as lpool,
            tc.tile_pool(name="trans", bufs=3) as tpool,
            tc.tile_pool(name="outbuf", bufs=2) as opool,
            tc.tile_pool(name="pst", bufs=4, space="PSUM") as pst_pool,
            tc.tile_pool(name="psg", bufs=3, space="PSUM") as psg_pool,
        ):
            ident = cpool.tile([128, 128], F16)
            nc.sync.dma_start(out=ident[:], in_=IDENT[:])

            for hb in range(0, h_run, NH):
                nat = {}
                for ti, (w0, tw) in enumerate(WTILES):
                    for tname, src in (("L", L), ("R", R)):
                        t = lpool.tile([tw, NH, C], F16, tag=f"nat{tname}{ti}")
                        # SWDGE casts fp32 -> fp16 inline during the load
                        nc.gpsimd.dma_start(
                            out=t[:],
                            in_=src[hb : hb + NH, w0 : w0 + tw, :].rearrange(
                                "h w c -> w h c"
                            ),
                        )
                        nat[(tname, ti)] = t

                outt = {
                    name: opool.tile([M, NH, Wn], F32, tag=name, name=f"t_{name}")
                    for name, _, M, Wn in BLOCKS
                }

                for hl in range(NH):
                    trs = {}
                    for tname in ("L", "R"):
                        tt = tpool.tile([C, W], F16, tag=f"T{tname}")
                        for ti, (w0, tw) in enumerate(WTILES):
                            ps = pst_pool.tile([C, 128], F32, tag="pst")
                            # transpose as a REGULAR f16 matmul (X^T = X.T @ I):
                            # pipelines at N cycles and keeps the PE HAM-warm
                            nc.tensor.matmul(
                                out=ps[:, :tw],
                                lhsT=nat[(tname, ti)][:tw, hl, :],
                                rhs=ident[:tw, :tw],
                                start=True,
                                stop=True,
                            )
                            nc.vector.tensor_copy(
                                out=tt[:, w0 : w0 + tw], in_=ps[:, :tw]
                            )
                        trs[tname] = tt

                    for name, u0, M, Wn in BLOCKS:
                        pg = psg_pool.tile([128, 175], F32, tag="psg")
                        nc.tensor.matmul(
                            out=pg[:M, :Wn],
                            lhsT=trs["R"][:, u0 : u0 + M],
                            rhs=trs["L"][:, u0 : u0 + Wn],
                            start=True,
                            stop=True,
                        )
                        nc.vector.tensor_copy(
                            out=outt[name][:, hl, :], in_=pg[:M, :Wn]
                        )

                for name, _, M, Wn in BLOCKS:
                    nc.sync.dma_start(
                        out=outs[name][hb : hb + NH].rearrange("h u j -> u h j"),
                        in_=outt[name][:],
                    )

    nc.compile()
    return nc


def _get_nc(h_run: int = H):
    if h_run not in _cache:
        _cache[h_run] = _build(h_run)
    return _cache[h_run]


def _reconstruct(results) -> np.ndarray:
    """Assemble [B, D, H, W] from the per-core band blocks."""
    XA = np.stack([r["OutA"] for r in results]).reshape(B, H, 128 * 175)
    XB = np.stack([r["OutB"] for r in results]).reshape(B, H, 128 * 175)
    XC = np.stack([r["OutC"] for r in results]).reshape(B, H, 64 * 64)
    out = np.zeros((B, D, H, W), np.float32)
    idx = np.arange(128) * 176
    for d in range(D):
        out[:, d, :, d : d + 128] = XA[:, :, idx + d]
        out[:, d, :, 128 + d : 256 + d] = XB[:, :, idx + d]
        nc_ = 64 - d
        out[:, d, :, 256 + d :] = XC[:, :, np.arange(nc_) * 65 + d]
    return out


def _run(L_full, R_full, h_run: int = H, trace: bool = False):
    L_full = np.ascontiguousarray(np.asarray(L_full), dtype=np.float32)
    R_full = np.ascontiguousarray(np.asarray(R_full), dtype=np.float32)
    assert L_full.shape == (B, H, W, C), L_full.shape
    nc = _get_nc(h_run)
    eye = np.eye(128, dtype=np.float16)
    in_maps = [{"L": L_full[b], "R": R_full[b], "IDENT": eye} for b in range(B)]
    res = run_bass_kernel_spmd(
        nc, in_maps, list(range(B)), trace=trace, trace_cores=[0] if trace else None
    )
    return _reconstruct(res.results), res


def kernel(L_corr, R_corr):
    out, _ = _run(L_corr, R_corr)
    return out


# revision 9
# speedup vs baseline: 1.5503x; 1.0696x over previous
"""Correlation-volume kernel for Trainium2 (8 NeuronCores, data-parallel over B).

corr[b, d, h, w] = sum_c L[b,h,w,c] * R[b,h,w-d,c], 0 <= d < 48, zero-padded w-d < 0.

Device strategy (per core = one batch):
  For each h row: transpose L/R rows into [C, W] layout on the PE (fp32r
  transpose-mode), then compute banded Gram tiles
      G^T[u, w] = sum_c R^T[c, u] * L^T[c, w]   (u-chunks of 128/128/64,
  moving = full 320-wide L^T row so fp32r streams at 1 cyc/row), copy the
  valid band window of each PSUM tile to SBUF and DMA rectangular blocks to
  DRAM. The host extracts the 48 diagonals (corr[d,h,w] = G^T[w-d, w]) while
  unsharding — that transpose is host-side glue, free for the device.
"""

import os
import sys

import numpy as np

for _p in (
    "/root/.axon_site",
    "/root/.axon_site/_ro/trn_rl_repo",
    "/root/.axon_site/_ro/pypackages",
    "/opt/trn_rl_repo",
    "/opt/pypackages",
):
    if os.path.isdir(_p) and _p not in sys.path:
        sys.path.append(_p)

import concourse.bacc as bacc
import concourse.mybir as mybir
import concourse.tile as tile
from concourse.bass_utils import run_bass_kernel_spmd

B, H, W, C, D = 8, 160, 320, 128, 48
NH = 10  # h rows per DMA batch
F32 = mybir.dt.float32
F16 = mybir.dt.float16

# u-chunks: (u0, M) with output window w in [u0, u0 + M + D - 2] clipped to W
WTILES = [(0, 128), (128, 128), (256, 64)]
BLOCKS = [  # (name, u0, M, window_width)
    ("OutA", 0, 128, 175),
    ("OutB", 128, 128, 175),
    ("OutC", 256, 64, 64),
]

_cache: dict = {}


def _build(h_run: int = H):
    nc = bacc.Bacc("TRN2", target_bir_lowering=False, debug=False, num_devices=B)
    L = nc.dram_tensor("L", [H, W, C], F32, kind="ExternalInput").ap()
    R = nc.dram_tensor("R", [H, W, C], F32, kind="ExternalInput").ap()
    IDENT = nc.dram_tensor("IDENT", [128, 128], F16, kind="ExternalInput").ap()
    outs = {
        name: nc.dram_tensor(name, [H, M, Wn], F32, kind="ExternalOutput").ap()
        for name, _, M, Wn in BLOCKS
    }

    with tile.TileContext(nc) as tc:
        with (
            tc.tile_pool(name="const", bufs=1) as cpool,
            tc.tile_pool(name="loads", bufs=2) as lpool,
            tc.tile_pool(name="trans", bufs=3) as tpool,
            tc.tile_pool(name="outbuf", bufs=2) as opool,
            tc.tile_pool(name="pst", bufs=4, space="PSUM") as pst_pool,
            tc.tile_pool(name="psg", bufs=3, space="PSUM") as psg_pool,
        ):
            ident = cpool.tile([128, 128], F16)
            nc.sync.dma_start(out=ident[:], in_=IDENT[:])

            for hb in range(0, h_run, NH):
                nat = {}
                for ti, (w0, tw) in enumerate(WTILES):
                    for tname, src in (("L", L), ("R", R)):
                        t = lpool.tile([tw, NH, C], F16, tag=f"nat{tname}{ti}")
                        # SWDGE casts fp32 -> fp16 inline during the load
                        nc.gpsimd.dma_start(
                            out=t[:],
                            in_=src[hb : hb + NH, w0 : w0 + tw, :].rearrange(
                                "h w c -> w h c"
                            ),
                        )
                        nat[(tname, ti)] = t


                gout = opool.tile([128, NH, 414], F32, tag="gout", name="t_gout")
                for hl in range(NH):
                    trs = {}
                    for tname in ("L", "R"):
                        tt = tpool.tile([C, W], F16, tag=f"T{tname}")
                        # all three w-tiles transpose into ONE PSUM bank so a
                        # single cast-copy drains it (DVE inst count 3x down)
                        ps = pst_pool.tile([C, W], F32, tag="pst")
                        for ti, (w0, tw) in enumerate(WTILES):
                            # transpose as a REGULAR f16 matmul (X^T = X.T @ I):
                            # pipelines at N cycles and keeps the PE HAM-warm
                            nc.tensor.matmul(
                                out=ps[:, w0 : w0 + tw],
                                lhsT=nat[(tname, ti)][:tw, hl, :],
                                rhs=ident[:tw, :tw],
                                start=True,
                                stop=True,
                            )
                        nc.vector.tensor_copy(out=tt[:], in_=ps[:])
                        trs[tname] = tt

                    # all three u-chunks land in ONE PSUM bank (175+175+64
                    # fp32 = 1656B <= 2KB); one copy drains them
                    pg = psg_pool.tile([128, 414], F32, tag="psg")
                    off = 0
                    for name, u0, M, Wn in BLOCKS:
                        nc.tensor.matmul(
                            out=pg[:M, off : off + Wn],
                            lhsT=trs["R"][:, u0 : u0 + M],
                            rhs=trs["L"][:, u0 : u0 + Wn],
                            start=True,
                            stop=True,
                        )
                        off += Wn
                    nc.vector.tensor_copy(out=gout[:, hl, :], in_=pg[:])

                off = 0
                for name, _, M, Wn in BLOCKS:
                    nc.sync.dma_start(
                        out=outs[name][hb : hb + NH].rearrange("h u j -> u h j"),
                        in_=gout[:M, :, off : off + Wn],
                    )
                    off += Wn

    nc.compile()
    return nc


def _get_nc(h_run: int = H):
    if h_run not in _cache:
        _cache[h_run] = _build(h_run)
    return _cache[h_run]


def _reconstruct(results) -> np.ndarray:
    """Assemble [B, D, H, W] from the per-core band blocks."""
    XA = np.stack([r["OutA"] for r in results]).reshape(B, H, 128 * 175)
    XB = np.stack([r["OutB"] for r in results]).reshape(B, H, 128 * 175)
    XC = np.stack([r["OutC"] for r in results]).reshape(B, H, 64 * 64)
    out = np.zeros((B, D, H, W), np.float32)
    idx = np.arange(128) * 176
    for d in range(D):
        out[:, d, :, d : d + 128] = XA[:, :, idx + d]
        out[:, d, :, 128 + d : 256 + d] = XB[:, :, idx + d]
        nc_ = 64 - d
        out[:, d, :, 256 + d :] = XC[:, :, np.arange(nc_) * 65 + d]
    return out


def _run(L_full, R_full, h_run: int = H, trace: bool = False):
    L_full = np.ascontiguousarray(np.asarray(L_full), dtype=np.float32)
    R_full = np.ascontiguousarray(np.asarray(R_full), dtype=np.float32)
    assert L_full.shape == (B, H, W, C), L_full.shape
    nc = _get_nc(h_run)
    eye = np.eye(128, dtype=np.float16)
    in_maps = [{"L": L_full[b], "R": R_full[b], "IDENT": eye} for b in range(B)]
    res = run_bass_kernel_spmd(
        nc, in_maps, list(range(B)), trace=trace, trace_cores=[0] if trace else None
    )
    return _reconstruct(res.results), res


def kernel(L_corr, R_corr):
    out, _ = _run(L_corr, R_corr)
    return out


# revision 12
# speedup vs baseline: 1.7271x; 1.1141x over previous
"""Correlation-volume kernel for Trainium2 (8 NeuronCores, data-parallel over B).

corr[b, d, h, w] = sum_c L[b,h,w,c] * R[b,h,w-d,c], 0 <= d < 48, zero-padded w-d < 0.

Device strategy (per core = one batch):
  - SWDGE loads cast fp32 -> fp16 inline; natural [w, h, c] tiles in SBUF.
  - Per h row, L/R rows are transposed to [C, W] via REGULAR f16 matmuls
    against an identity (pipelines at N cycles, keeps the PE HAM-warm).
  - Banded Gram tiles G^T[u, w] = sum_c R^T[c,u] * L^T[c,w] in u-chunks of
    64, two h rows packed onto the 128 PSUM partitions via col-tiling
    (tile_position=(0,64) for the odd row). Valid band window w in
    [u0, u0+110] per chunk -> 5 chunks fill one PSUM bank [128, 508].
  - One DVE copy drains each h-pair into a padded [5, 112]-per-chunk SBUF
    block; one 1.4 MB DMA per NH rows writes DRAM.
  - Host extracts the 48 diagonals (corr[d,h,w] = G^T[w-d, w]) while
    unsharding: host-side glue, free for the device.
"""

import os
import sys

import numpy as np

for _p in (
    "/root/.axon_site",
    "/root/.axon_site/_ro/trn_rl_repo",
    "/root/.axon_site/_ro/pypackages",
    "/opt/trn_rl_repo",
    "/opt/pypackages",
):
    if os.path.isdir(_p) and _p not in sys.path:
        sys.path.append(_p)

import concourse.bacc as bacc
import concourse.mybir as mybir
import concourse.tile as tile
from concourse.bass_utils import run_bass_kernel_spmd

B, H, W, C, D = 8, 160, 320, 128, 48
NH = 10  # h rows per DMA batch (even)
F32 = mybir.dt.float32
F16 = mybir.dt.float16

WTILES = [(0, 128), (128, 128), (256, 64)]
# u-chunks of 64: (u0, window width); window w in [u0, min(u0+64+47, W))
CHUNKS = [(0, 111), (64, 111), (128, 111), (192, 111), (256, 64)]
NK = len(CHUNKS)
PW = 112  # padded per-chunk width in the output blocks
PSW = sum(wn for _, wn in CHUNKS)  # 508 fp32 = 2032B, fits one PSUM bank

_cache: dict = {}


def _build(h_run: int = H):
    nc = bacc.Bacc("TRN2", target_bir_lowering=False, debug=False, num_devices=B)
    L = nc.dram_tensor("L", [H, W, C], F32, kind="ExternalInput").ap()
    R = nc.dram_tensor("R", [H, W, C], F32, kind="ExternalInput").ap()
    IDENT = nc.dram_tensor("IDENT", [128, 128], F16, kind="ExternalInput").ap()
    # [(p,u), hh, k, j]: h = 2*hh + p, w = u0_k + j, corr[j-u, h, w]
    OUT = nc.dram_tensor(
        "OUT", [128, H // 2, NK, PW], F32, kind="ExternalOutput"
    ).ap()

    with tile.TileContext(nc) as tc:
        with (
            tc.tile_pool(name="const", bufs=1) as cpool,
            tc.tile_pool(name="loads", bufs=2) as lpool,
            tc.tile_pool(name="trans", bufs=4) as tpool,
            tc.tile_pool(name="outbuf", bufs=2) as opool,
            tc.tile_pool(name="pst", bufs=4, space="PSUM") as pst_pool,
            tc.tile_pool(name="psg", bufs=3, space="PSUM") as psg_pool,
        ):
            ident = cpool.tile([128, 128], F16)
            nc.sync.dma_start(out=ident[:], in_=IDENT[:])

            for hb in range(0, h_run, NH):
                nat = {}
                for ti, (w0, tw) in enumerate(WTILES):
                    for tname, src in (("L", L), ("R", R)):
                        t = lpool.tile([tw, NH, C], F16, tag=f"nat{tname}{ti}")
                        # SWDGE casts fp32 -> fp16 inline during the load
                        nc.gpsimd.dma_start(
                            out=t[:],
                            in_=src[hb : hb + NH, w0 : w0 + tw, :].rearrange(
                                "h w c -> w h c"
                            ),
                        )
                        nat[(tname, ti)] = t

                gout = opool.tile([128, NH // 2, NK, PW], F32, tag="gout")

                for hp in range(NH // 2):
                    trs = {}
                    for p in range(2):
                        hl = 2 * hp + p
                        for tname in ("L", "R"):
                            ps = pst_pool.tile([C, W], F32, tag="pst")
                            for ti, (w0, tw) in enumerate(WTILES):
                                nc.tensor.matmul(
                                    out=ps[:, w0 : w0 + tw],
                                    lhsT=nat[(tname, ti)][:tw, hl, :],
                                    rhs=ident[:tw, :tw],
                                    start=True,
                                    stop=True,
                                )
                            tt = tpool.tile([C, W], F16, tag=f"T{tname}{p}")
                            nc.vector.tensor_copy(out=tt[:], in_=ps[:])
                            trs[(tname, p)] = tt

                    pg = psg_pool.tile([128, PSW], F32, tag="psg")
                    for p in range(2):
                        off = 0
                        for u0, wn in CHUNKS:
                            nc.tensor.matmul(
                                out=pg[64 * p : 64 * p + 64, off : off + wn],
                                lhsT=trs[("R", p)][:, u0 : u0 + 64],
                                rhs=trs[("L", p)][:, u0 : u0 + wn],
                                start=True,
                                stop=True,
                                tile_position=(0, 64 * p),
                            )
                            off += wn
                    # drain the pair: 4x111 into padded 112-stride slots + tail 64
                    nc.vector.tensor_copy(
                        out=gout[:, hp, 0 : NK - 1, 0:111],
                        in_=pg[:, 0 : 4 * 111].rearrange("p (k j) -> p k j", j=111),
                    )
                    nc.vector.tensor_copy(
                        out=gout[:, hp, NK - 1, 0:64], in_=pg[:, 4 * 111 : PSW]
                    )

                nc.sync.dma_start(
                    out=OUT[:, hb // 2 : hb // 2 + NH // 2, :, :],
                    in_=gout[:],
                )

    nc.compile()
    return nc


def _get_nc(h_run: int = H):
    if h_run not in _cache:
        _cache[h_run] = _build(h_run)
    return _cache[h_run]


def _reconstruct(results) -> np.ndarray:
    """Assemble [B, D, H, W] from the per-core band blocks."""
    # X[b, (p,u), hh, k, j] = corr[b, j-u, 2hh+p, u0_k + j]
    X = np.stack([r["OUT"] for r in results])  # [B, 128, H/2, NK, PW]
    # -> [B, k, hh, p, u, j] flat over (u, j)
    Xr = X.reshape(B, 2, 64, H // 2, NK, PW).transpose(0, 4, 3, 1, 2, 5)
    Xf = np.ascontiguousarray(Xr).reshape(B, NK, H // 2, 2, 64 * PW)
    out = np.zeros((B, D, H, W), np.float32)
    u = np.arange(64)
    for d in range(D):
        idx = u * (PW + 1) + d
        for k, (u0, wn) in enumerate(CHUNKS):
            nu = min(64, W - u0 - d)
            v = Xf[:, k][:, :, :, idx[:nu]]  # [B, H/2, 2, nu]
            out[:, d, :, u0 + d : u0 + d + nu] = v.reshape(B, H, nu)
    return out


def _run(L_full, R_full, h_run: int = H, trace: bool = False):
    L_full = np.ascontiguousarray(np.asarray(L_full), dtype=np.float32)
    R_full = np.ascontiguousarray(np.asarray(R_full), dtype=np.float32)
    assert L_full.shape == (B, H, W, C), L_full.shape
    nc = _get_nc(h_run)
    eye = np.eye(128, dtype=np.float16)
    in_maps = [{"L": L_full[b], "R": R_full[b], "IDENT": eye} for b in range(B)]
    res = run_bass_kernel_spmd(
        nc, in_maps, list(range(B)), trace=trace, trace_cores=[0] if trace else None
    )
    return _reconstruct(res.results), res


def kernel(L_corr, R_corr):
    out, _ = _run(L_corr, R_corr)
    return out
